# revision 15
# baseline (speedup 1.0000x reference)
"""Trainium2 Bass kernel for nn_ClassificationHead: LayerNorm -> Linear(1024,256) -> GELU -> Linear(256,2).

Data-parallel over 8 NeuronCores: each core processes 8192 rows (64 tiles of
128) in 16 blocks of 512; the tiny weights are replicated. M-major dataflow
(stationary = x chunk, moving = weights), with the LayerNorm folded into the
matmul via a rank-2 correction.

Per-core pipeline, per 512-row block:
  1. Two DMA rings prefetch (2 blocks ahead): x K-major bf16
     [128, G, KC, 128] (sync ring) and an fp8(e4m3) copy packed for
     DoubleRow [128, G, 4, 2, 144] with a ones column at 128 (gpsimd ring).
  2. Per 128-row tile, TensorE runs 4 fp8-DoubleRow Gram matmuls (2 chunks
     per instruction, 2x fp8 rate) -> psum [128, 129]: diag = sum x^2,
     col 128 = rowsum. DVE extracts the diagonal (identity-mask STT with
     accumulate); ACT writes -mu = -rowsum/D straight into the BM pack.
  3. GpSimd computes V = SS/D + eps - mu^2; DVE runs the bit-trick Newton
     rsqrt -> g (fp32, kept for the GELU scale) and rhat = V*g -> BM
     [128, (q: -mu, rhat)] bf16.
  4. TensorE runs mm1 per tile: 8 accumulating matmuls (stationary = x
     chunk, moving = W1' [128, 256]); one PE transpose flips BM into
     BMT [8, 128] (interleaved with mm1 so the ACT psum->sbuf copy hides).
  5. TensorE adds the rank-2 LN correction with ONE K=8 matmul per tile:
     stationary = BMT, moving = per-tile [8, 256] strip holding (s1, c1) at
     rows (2q, 2q+1), zeros elsewhere -> psum = (LN(x)@W1'+b1)/g.
  6. ACT evaluates exact GELU with per-partition scale g -> h bf16; DVE
     computes h @ W2 one block late (2 masked-reduce STTs per tile, ordered
     after the next block's stats chain) and adds b2.
  7. The [8192, 2] fp32 result is written back in two contiguous drains.

Host-side prep is layout-only + tiny O(1MB) weight folding: W1' = ln_w*W1,
s1 = colsum(W1'), c1 = ln_b@W1 + b1, bf16/fp8 casts of x. Rows are permuted
(row 64m+4u+q at partition m) so input and output DMAs stay dense.
"""
import sys

sys.path.insert(0, "/opt/trn_rl_repo")
sys.path.insert(0, "/root/.axon_site")

import numpy as np
import ml_dtypes

N_CORES = 8
BATCH = 65536
D = 1024
H = 256
OUT = 2
RPC = BATCH // N_CORES  # rows per core
NT = RPC // 128         # 128-row tiles per core
KC = D // 128           # contraction chunks
KCP = KC // 2           # fp8 DoubleRow chunk pairs
G = 4                   # tiles per block (512 rows)
NB = NT // G            # blocks per core
EPS = 1e-5
MAGIC = 0x5F3759DF
F8PAD = 144             # fp8 pair stride (16B aligned): cols 0:128 x, 128 ones, rest 0

_cache = {}


def _bf16(a):
    return np.asarray(a, dtype=ml_dtypes.bfloat16)


def _build():
    import concourse.bacc as bacc
    import concourse.mybir as mybir
    from concourse import tile

    f32 = mybir.dt.float32
    i32 = mybir.dt.int32
    bf16 = mybir.dt.bfloat16
    fp8 = mybir.dt.float8e4
    AF = mybir.ActivationFunctionType
    ALU = mybir.AluOpType
    PM = mybir.MatmulPerfMode

    nc = bacc.Bacc(None, target_bir_lowering=False, debug=False)

    xt_in = nc.dram_tensor("xt", [128, NB, G, KC, 128], bf16, kind="ExternalInput")
    x8_in = nc.dram_tensor("x8", [128, NB, G, KCP, 2, F8PAD], fp8, kind="ExternalInput")
    w1_in = nc.dram_tensor("w1p", [128, KC, H], bf16, kind="ExternalInput")
    sc_in = nc.dram_tensor("scq", [8, G, H], bf16, kind="ExternalInput")
    w2_in = nc.dram_tensor("w2rep", [128, OUT, H], bf16, kind="ExternalInput")
    b2_in = nc.dram_tensor("b2g", [128, G * OUT], f32, kind="ExternalInput")
    idb_in = nc.dram_tensor("identb", [128, 128], bf16, kind="ExternalInput")
    idf_in = nc.dram_tensor("identf", [128, 128], f32, kind="ExternalInput")
    y_out = nc.dram_tensor("y", [RPC, OUT], f32, kind="ExternalOutput")
    y_v = y_out.rearrange("(p t) c -> p t c", p=128)

    with tile.TileContext(nc) as tc:
        with (
            tc.tile_pool(name="wpool", bufs=1) as wp,
            tc.tile_pool(name="xtp", bufs=3) as xtp,
            tc.tile_pool(name="x8p", bufs=3) as x8p,
            tc.tile_pool(name="scrp", bufs=2) as scrp,
            tc.tile_pool(name="statp", bufs=2) as statp,
            tc.tile_pool(name="bmp", bufs=2) as bmp,
            tc.tile_pool(name="bmtp", bufs=2) as bmtp,
            tc.tile_pool(name="hbp", bufs=9) as hbp,
            tc.tile_pool(name="s2p", bufs=3) as s2p,
            tc.tile_pool(name="outp", bufs=1) as outp,
            tc.tile_pool(name="pszp", bufs=5, space="PSUM") as pszp,
            tc.tile_pool(name="psgp", bufs=2, space="PSUM") as psgp,
            tc.tile_pool(name="pstp", bufs=1, space="PSUM") as pstp,
        ):
            # Prefetched input streams, 2 blocks ahead.
            xts, x8s = [], []

            def fetch_x8(u):
                t = x8p.tile([128, G, KCP, 2, F8PAD], fp8, tag="x8")
                nc.gpsimd.dma_start(t[:], x8_in[:, u])
                x8s.append(t)

            def fetch_xt(u):
                t = xtp.tile([128, G, KC, 128], bf16, tag="xt")
                nc.sync.dma_start(t[:, 0:2], xt_in[:, u, 0:2])
                nc.sync.dma_start(t[:, 2:4], xt_in[:, u, 2:4])
                xts.append(t)

            fetch_x8(0)
            idbsb = wp.tile([128, 128], bf16)
            nc.sync.dma_start(idbsb[:], idb_in[:])
            w1sb = wp.tile([128, KC, H], bf16)
            nc.sync.dma_start(w1sb[:], w1_in[:])
            fetch_xt(0)
            idfsb = wp.tile([128, 128], f32)
            nc.scalar.dma_start(idfsb[:], idf_in[:])
            scsb = wp.tile([8, G, H], bf16)
            nc.scalar.dma_start(scsb[:], sc_in[:])
            w2sb = wp.tile([128, OUT, H], bf16)
            nc.scalar.dma_start(w2sb[:], w2_in[:])
            b2sb = wp.tile([128, G * OUT], f32)
            nc.scalar.dma_start(b2sb[:], b2_in[:])
            # Gate block-1's fp8 prefetch behind w1's arrival so the head's
            # critical DMAs (w1, xt0) get the full aggregate bandwidth.
            gate = wp.tile([1, 1], bf16)
            nc.gpsimd.tensor_copy(gate[:], w1sb[0:1, 0, 0:1])
            fetch_x8(1)
            fetch_xt(1)

            outsb = outp.tile([128, NT, OUT], f32)

            # Warmup junk matmuls on the identity: open the PE clock gate
            # while the first x8 DMA is in flight. Results never consumed.
            warm = psgp.tile([128, F8PAD - 15], f32, tag="psg")
            for _ in range(20):
                nc.tensor.matmul(warm[:, 0:128], idbsb[:], idbsb[:],
                                 start=True, stop=True, skip_group_check=True)

            def w2_tile(hb, OB, q):
                for c in range(OUT):
                    scr2 = s2p.tile([128, H], bf16, tag=f"scr2{c}")
                    nc.vector.scalar_tensor_tensor(
                        scr2[:], hb[:], 1.0, w2sb[:, c, :],
                        ALU.mult, ALU.mult, accum_out=OB[:, q, c : c + 1],
                    )

            prev = None  # (u, hb_list, OB) of previous block
            for u in range(NB):
                if u + 2 < NB:
                    fetch_x8(u + 2)
                    fetch_xt(u + 2)
                x8g = x8s[u]
                xtg = xts[u]

                BM = bmp.tile([128, 2 * G], bf16, tag="BM")
                SS = statp.tile([128, G], f32, tag="SS")

                # fp8 DoubleRow grams + extracts (diag -> SS on DVE, -mu -> BM
                # on ACT).
                for q in range(G):
                    psg = psgp.tile([128, F8PAD - 15], f32, tag="psg")
                    for j in range(KCP):
                        nc.tensor.matmul(
                            psg[:, 0:129], x8g[:, q, j, :, 0:128],
                            x8g[:, q, j, :, 0:129],
                            start=(j == 0), stop=(j == KCP - 1),
                            perf_mode=PM.DoubleRow,
                        )
                    scr = scrp.tile([128, 128], f32, tag="scr")
                    nc.vector.scalar_tensor_tensor(
                        scr[:], idfsb[:], 1.0, psg[:, 0:128],
                        ALU.mult, ALU.mult, accum_out=SS[:, q : q + 1],
                    )
                    nc.scalar.activation(
                        BM[:, 2 * q : 2 * q + 1], psg[:, 128:129], AF.Copy,
                        bias=0.0, scale=-1.0 / D,
                    )

                # V = SS/D + eps - mu^2 on GpSimd (idle engine); bit-trick
                # Newton rsqrt on DVE -> g (fp32, GELU scale) and rhat = V*g.
                musq = statp.tile([128, G], f32, tag="musq")
                nc.gpsimd.tensor_tensor(musq[:], BM[:, 0 : 2 * G : 2], BM[:, 0 : 2 * G : 2], ALU.mult)
                A1 = statp.tile([128, G], f32, tag="A1")
                nc.gpsimd.tensor_scalar(A1[:], SS[:], 1.0 / D, EPS, ALU.mult, ALU.add)
                V = statp.tile([128, G], f32, tag="V")
                nc.gpsimd.tensor_tensor(V[:], A1[:], musq[:], ALU.subtract)
                Y = statp.tile([128, G], f32, tag="Y")
                T = statp.tile([128, G], f32, tag="T")
                nc.vector.tensor_scalar(T[:].bitcast(i32), V[:].bitcast(i32), 1, None, ALU.logical_shift_right)
                nc.vector.tensor_scalar(Y[:].bitcast(i32), T[:].bitcast(i32), -1, MAGIC, ALU.mult, ALU.add)
                nc.vector.tensor_tensor(T[:], V[:], Y[:], ALU.mult)
                nc.vector.tensor_tensor(T[:], T[:], Y[:], ALU.mult)
                nc.vector.tensor_scalar(T[:], T[:], -0.5, 1.5, ALU.mult, ALU.add)
                nc.vector.tensor_tensor(Y[:], Y[:], T[:], ALU.mult)
                nc.vector.tensor_tensor(BM[:, 1 : 2 * G : 2], V[:], Y[:], ALU.mult)

                # mm1 (tile 0), then the BM transpose (its ACT copy hides
                # under the remaining mm1 tiles), then mm1 tiles 1..3.
                pszs = []

                def mm1(q):
                    pszg = pszp.tile([128, H], f32, tag="pszg")
                    for k in range(KC):
                        nc.tensor.matmul(
                            pszg[:], xtg[:, q, k, :], w1sb[:, k, :],
                            start=(k == 0), stop=False,
                        )
                    pszs.append(pszg)

                mm1(0)
                pst = pstp.tile([8, 128], bf16, tag="pst")
                nc.tensor.transpose(pst[:], BM[:], idbsb[:])
                for q in range(1, G):
                    mm1(q)
                bmt = bmtp.tile([8, 128], bf16, tag="bmt")
                nc.scalar.copy(bmt[:], pst[:])

                # Previous block's W2 goes after this block's stats chain in
                # the DVE queue.
                if prev is not None:
                    up, hbl, OBp = prev
                    for q in range(G):
                        w2_tile(hbl[q], OBp, q)
                    nc.vector.tensor_add(
                        outsb[:, up * G : (up + 1) * G, :].opt(),
                        OBp[:].opt(),
                        b2sb[:].rearrange("p (q c) -> p q c", c=OUT),
                    )

                # Rank-2 LN correction (one K=8 matmul per tile) + GELU.
                last = u == NB - 1
                OB = statp.tile([128, G, OUT], f32, tag="OB")
                hbs = []
                for q in range(G):
                    nc.tensor.matmul(
                        pszs[q][:], bmt[0:8, :], scsb[:, q, :],
                        start=False, stop=True, skip_group_check=True,
                    )
                    hb = hbp.tile([128, H], bf16, tag="hb")
                    nc.scalar.activation(
                        hb[:], pszs[q][:], AF.Gelu, bias=0.0, scale=Y[:, q : q + 1]
                    )
                    hbs.append(hb)
                    if last:
                        w2_tile(hb, OB, q)

                prev = (u, hbs, OB)
                if last:
                    nc.vector.tensor_add(
                        outsb[:, u * G : (u + 1) * G, :].opt(),
                        OB[:].opt(),
                        b2sb[:].rearrange("p (q c) -> p q c", c=OUT),
                    )
                if u == NB // 2:
                    nc.scalar.dma_start(y_v[:, 0 : NT // 2], outsb[:, 0 : NT // 2])
                elif u == NB - 2:
                    nc.scalar.dma_start(y_v[:, NT // 2 : 13 * G], outsb[:, NT // 2 : 13 * G])

            nc.scalar.dma_start(y_v[:, 13 * G :], outsb[:, 13 * G :])

    nc.finalize()
    return nc


def _get_nc():
    if "nc" not in _cache:
        _cache["nc"] = _build()
    return _cache["nc"]


def _prep_weights(ln_w, ln_b, W1, b1, W2, b2):
    W1p = ln_w[:, None] * W1                      # [1024, 256]
    s1 = W1p.sum(axis=0)                          # [256]
    c1 = ln_b @ W1 + b1                           # [256]
    # Rank-2 moving strips: tile q reads rows (2q, 2q+1) = (s1, c1); other
    # rows pair with other tiles' stats and must be zero.
    sc = np.zeros((8, G, H), np.float32)
    for q in range(G):
        sc[2 * q, q] = s1
        sc[2 * q + 1, q] = c1
    return {
        "w1p": _bf16(W1p.reshape(KC, 128, H).transpose(1, 0, 2)),
        "scq": _bf16(sc),
        "w2rep": _bf16(np.broadcast_to(W2.T, (128, OUT, H))),
        "b2g": np.broadcast_to(np.tile(b2, G), (128, G * OUT)).astype(np.float32).copy(),
        "identb": _bf16(np.eye(128)),
        "identf": np.eye(128, dtype=np.float32),
    }


def _shard_input(x_core):
    """fp32 [8192, 1024] -> bf16 K-major [128, NB, G, KC, 128] with rows
    permuted so tile t=4u+q, partition m holds row 64m+4u+q (dense input and
    output DMAs), plus the fp8 DoubleRow-padded copy with a ones column."""
    xr = x_core.reshape(128, NB, G, KC, 128)        # [m, u, q, c, p]
    xt = np.ascontiguousarray(xr.transpose(4, 1, 2, 3, 0))  # [p, u, q, c, m]
    x8 = np.zeros((128, NB, G, KC, F8PAD), ml_dtypes.float8_e4m3fn)
    x8[..., 0:128] = np.asarray(xt, dtype=ml_dtypes.float8_e4m3fn)
    x8[..., 128] = 1.0
    return _bf16(xt), x8.reshape(128, NB, G, KCP, 2, F8PAD)


def _make_in_maps(embedding, ln_w, ln_b, W1, b1, W2, b2):
    embedding = np.asarray(embedding, dtype=np.float32)
    weights = _prep_weights(
        np.asarray(ln_w, dtype=np.float32), np.asarray(ln_b, dtype=np.float32),
        np.asarray(W1, dtype=np.float32), np.asarray(b1, dtype=np.float32),
        np.asarray(W2, dtype=np.float32), np.asarray(b2, dtype=np.float32),
    )
    maps = []
    for c in range(N_CORES):
        xt, x8 = _shard_input(embedding[c * RPC : (c + 1) * RPC])
        maps.append({"xt": xt, "x8": x8, **weights})
    return maps


def kernel(embedding, ln_w, ln_b, W1, b1, W2, b2):
    from concourse.bass_utils import run_bass_kernel_spmd

    in_maps = _make_in_maps(embedding, ln_w, ln_b, W1, b1, W2, b2)
    nc = _get_nc()
    res = run_bass_kernel_spmd(nc, in_maps, core_ids=list(range(N_CORES)))
    out = np.concatenate([res.results[c]["y"] for c in range(N_CORES)], axis=0)
    return out.astype(np.float32)


# revision 17
# speedup vs baseline: 1.0530x; 1.0530x over previous
"""Trainium2 Bass kernel for nn_ClassificationHead: LayerNorm -> Linear(1024,256) -> GELU -> Linear(256,2).

Data-parallel over 8 NeuronCores: each core processes 8192 rows (64 tiles of
128) in 16 blocks of 512; the tiny weights are replicated. M-major dataflow
(stationary = x chunk, moving = weights), with the LayerNorm folded into the
matmul via a rank-2 correction.

Per-core pipeline, per 512-row block:
  1. Two DMA rings prefetch (2 blocks ahead): x K-major bf16
     [128, G, KC, 128] (sync ring) and an fp8(e4m3) copy packed for
     DoubleRow [128, G, 4, 2, 144] with a ones column at 128 (gpsimd ring).
  2. Per 128-row tile, TensorE runs 4 fp8-DoubleRow Gram matmuls (2 chunks
     per instruction, 2x fp8 rate) -> psum [128, 129]: diag = sum x^2,
     col 128 = rowsum. DVE extracts the diagonal (identity-mask STT with
     accumulate); ACT writes -mu = -rowsum/D straight into the BM pack.
  3. GpSimd computes V = SS/D + eps - mu^2; DVE runs the bit-trick Newton
     rsqrt -> g (fp32, kept for the GELU scale) and rhat = V*g -> BM
     [128, (q: -mu, rhat)] bf16.
  4. TensorE runs mm1 per tile: 8 accumulating matmuls (stationary = x
     chunk, moving = W1' [128, 256]); one PE transpose flips BM into
     BMT [8, 128] (interleaved with mm1 so the ACT psum->sbuf copy hides).
  5. TensorE adds the rank-2 LN correction with ONE K=8 matmul per tile:
     stationary = BMT, moving = per-tile [8, 256] strip holding (s1, c1) at
     rows (2q, 2q+1), zeros elsewhere -> psum = (LN(x)@W1'+b1)/g.
  6. ACT evaluates exact GELU with per-partition scale g -> h bf16; DVE
     computes h @ W2 one block late (2 masked-reduce STTs per tile, ordered
     after the next block's stats chain) and adds b2.
  7. The [8192, 2] fp32 result is written back in two contiguous drains.

Host-side prep is layout-only + tiny O(1MB) weight folding: W1' = ln_w*W1,
s1 = colsum(W1'), c1 = ln_b@W1 + b1, bf16/fp8 casts of x. Rows are permuted
(row 64m+4u+q at partition m) so input and output DMAs stay dense.
"""
import sys

sys.path.insert(0, "/opt/trn_rl_repo")
sys.path.insert(0, "/root/.axon_site")

import numpy as np
import ml_dtypes

N_CORES = 8
BATCH = 65536
D = 1024
H = 256
OUT = 2
RPC = BATCH // N_CORES  # rows per core
NT = RPC // 128         # 128-row tiles per core
KC = D // 128           # contraction chunks
KCP = KC // 2           # fp8 DoubleRow chunk pairs
G = 4                   # tiles per block (512 rows)
NB = NT // G            # blocks per core
EPS = 1e-5
MAGIC = 0x5F3759DF
F8PAD = 144             # fp8 pair stride (16B aligned): cols 0:128 x, 128 ones, rest 0

_cache = {}


def _bf16(a):
    return np.asarray(a, dtype=ml_dtypes.bfloat16)


def _build():
    import concourse.bacc as bacc
    import concourse.mybir as mybir
    from concourse import tile

    f32 = mybir.dt.float32
    i32 = mybir.dt.int32
    bf16 = mybir.dt.bfloat16
    fp8 = mybir.dt.float8e4
    AF = mybir.ActivationFunctionType
    ALU = mybir.AluOpType
    PM = mybir.MatmulPerfMode

    nc = bacc.Bacc(None, target_bir_lowering=False, debug=False)

    xt_in = nc.dram_tensor("xt", [128, NB, G, KC, 128], bf16, kind="ExternalInput")
    x8_in = nc.dram_tensor("x8", [128, NB, G, KCP, 2, F8PAD], fp8, kind="ExternalInput")
    w1_in = nc.dram_tensor("w1p", [128, KC, H], bf16, kind="ExternalInput")
    sc_in = nc.dram_tensor("scq", [8, G, H], bf16, kind="ExternalInput")
    w2_in = nc.dram_tensor("w2rep", [128, OUT, H], bf16, kind="ExternalInput")
    b2_in = nc.dram_tensor("b2g", [128, G * OUT], f32, kind="ExternalInput")
    idb_in = nc.dram_tensor("identb", [128, 128], bf16, kind="ExternalInput")
    idf_in = nc.dram_tensor("identf", [128, 128], f32, kind="ExternalInput")
    y_out = nc.dram_tensor("y", [RPC, OUT], f32, kind="ExternalOutput")
    y_v = y_out.rearrange("(p t) c -> p t c", p=128)

    with tile.TileContext(nc) as tc:
        with (
            tc.tile_pool(name="wpool", bufs=1) as wp,
            tc.tile_pool(name="xtp", bufs=3) as xtp,
            tc.tile_pool(name="x8p", bufs=3) as x8p,
            tc.tile_pool(name="scrp", bufs=2) as scrp,
            tc.tile_pool(name="statp", bufs=2) as statp,
            tc.tile_pool(name="bmp", bufs=2) as bmp,
            tc.tile_pool(name="bmtp", bufs=2) as bmtp,
            tc.tile_pool(name="hbp", bufs=9) as hbp,
            tc.tile_pool(name="s2p", bufs=3) as s2p,
            tc.tile_pool(name="outp", bufs=1) as outp,
            tc.tile_pool(name="pszp", bufs=5, space="PSUM") as pszp,
            tc.tile_pool(name="psgp", bufs=2, space="PSUM") as psgp,
            tc.tile_pool(name="pstp", bufs=1, space="PSUM") as pstp,
        ):
            # Prefetched input streams, 2 blocks ahead.
            xts, x8s = [], []

            def fetch_x8(u):
                t = x8p.tile([128, G, KCP, 2, F8PAD], fp8, tag="x8")
                nc.gpsimd.dma_start(t[:], x8_in[:, u])
                x8s.append(t)

            def fetch_xt(u):
                t = xtp.tile([128, G, KC, 128], bf16, tag="xt")
                nc.sync.dma_start(t[:, 0:2], xt_in[:, u, 0:2])
                nc.sync.dma_start(t[:, 2:4], xt_in[:, u, 2:4])
                xts.append(t)

            fetch_x8(0)
            idbsb = wp.tile([128, 128], bf16)
            nc.sync.dma_start(idbsb[:], idb_in[:])
            fetch_xt(0)
            w1sb = wp.tile([128, KC, H], bf16)
            nc.sync.dma_start(w1sb[:], w1_in[:])
            idfsb = wp.tile([128, 128], f32)
            nc.scalar.dma_start(idfsb[:], idf_in[:])
            scsb = wp.tile([8, G, H], bf16)
            nc.scalar.dma_start(scsb[:], sc_in[:])
            w2sb = wp.tile([128, OUT, H], bf16)
            nc.scalar.dma_start(w2sb[:], w2_in[:])
            b2sb = wp.tile([128, G * OUT], f32)
            nc.scalar.dma_start(b2sb[:], b2_in[:])
            fetch_x8(1)
            fetch_xt(1)

            outsb = outp.tile([128, NT, OUT], f32)

            # Warmup junk matmuls on the identity: open the PE clock gate
            # while the first x8 DMA is in flight. Results never consumed.
            warm = psgp.tile([128, F8PAD - 15], f32, tag="psg")
            for _ in range(22):
                nc.tensor.matmul(warm[:, 0:128], idbsb[:], idbsb[:],
                                 start=True, stop=True, skip_group_check=True)

            def w2_tile(hb, OB, q):
                for c in range(OUT):
                    scr2 = s2p.tile([128, H], bf16, tag=f"scr2{c}")
                    nc.vector.scalar_tensor_tensor(
                        scr2[:], hb[:], 1.0, w2sb[:, c, :],
                        ALU.mult, ALU.mult, accum_out=OB[:, q, c : c + 1],
                    )

            prev = None  # (u, hb_list, OB) of previous block
            for u in range(NB):
                if u + 2 < NB:
                    fetch_x8(u + 2)
                    fetch_xt(u + 2)
                x8g = x8s[u]
                xtg = xts[u]

                BM = bmp.tile([128, 2 * G], bf16, tag="BM")
                SS = statp.tile([128, G], f32, tag="SS")

                # fp8 DoubleRow grams + extracts (diag -> SS on DVE, -mu -> BM
                # on ACT).
                for q in range(G):
                    psg = psgp.tile([128, F8PAD - 15], f32, tag="psg")
                    for j in range(KCP):
                        nc.tensor.matmul(
                            psg[:, 0:129], x8g[:, q, j, :, 0:128],
                            x8g[:, q, j, :, 0:129],
                            start=(j == 0), stop=(j == KCP - 1),
                            perf_mode=PM.DoubleRow,
                        )
                    scr = scrp.tile([128, 128], f32, tag="scr")
                    nc.vector.scalar_tensor_tensor(
                        scr[:], idfsb[:], 1.0, psg[:, 0:128],
                        ALU.mult, ALU.mult, accum_out=SS[:, q : q + 1],
                    )
                    nc.scalar.activation(
                        BM[:, 2 * q : 2 * q + 1], psg[:, 128:129], AF.Copy,
                        bias=0.0, scale=-1.0 / D,
                    )

                # V = SS/D + eps - mu^2 on GpSimd (idle engine); bit-trick
                # Newton rsqrt on DVE -> g (fp32, GELU scale) and rhat = V*g.
                musq = statp.tile([128, G], f32, tag="musq")
                nc.gpsimd.tensor_tensor(musq[:], BM[:, 0 : 2 * G : 2], BM[:, 0 : 2 * G : 2], ALU.mult)
                A1 = statp.tile([128, G], f32, tag="A1")
                nc.gpsimd.tensor_scalar(A1[:], SS[:], 1.0 / D, EPS, ALU.mult, ALU.add)
                V = statp.tile([128, G], f32, tag="V")
                nc.gpsimd.tensor_tensor(V[:], A1[:], musq[:], ALU.subtract)
                Y = statp.tile([128, G], f32, tag="Y")
                T = statp.tile([128, G], f32, tag="T")
                nc.vector.tensor_scalar(T[:].bitcast(i32), V[:].bitcast(i32), 1, None, ALU.logical_shift_right)
                nc.vector.tensor_scalar(Y[:].bitcast(i32), T[:].bitcast(i32), -1, MAGIC, ALU.mult, ALU.add)
                nc.vector.tensor_tensor(T[:], V[:], Y[:], ALU.mult)
                nc.vector.tensor_tensor(T[:], T[:], Y[:], ALU.mult)
                nc.vector.tensor_scalar(T[:], T[:], -0.5, 1.5, ALU.mult, ALU.add)
                nc.vector.tensor_tensor(Y[:], Y[:], T[:], ALU.mult)
                nc.vector.tensor_tensor(BM[:, 1 : 2 * G : 2], V[:], Y[:], ALU.mult)

                # mm1 (tile 0), then the BM transpose (its ACT copy hides
                # under the remaining mm1 tiles), then mm1 tiles 1..3.
                pszs = []

                def mm1(q):
                    pszg = pszp.tile([128, H], f32, tag="pszg")
                    for k in range(KC):
                        nc.tensor.matmul(
                            pszg[:], xtg[:, q, k, :], w1sb[:, k, :],
                            start=(k == 0), stop=False,
                        )
                    pszs.append(pszg)

                mm1(0)
                pst = pstp.tile([8, 128], bf16, tag="pst")
                nc.tensor.transpose(pst[:], BM[:], idbsb[:])
                for q in range(1, G):
                    mm1(q)
                bmt = bmtp.tile([8, 128], bf16, tag="bmt")
                nc.scalar.copy(bmt[:], pst[:])

                # Previous block's W2 goes after this block's stats chain in
                # the DVE queue.
                if prev is not None:
                    up, hbl, OBp = prev
                    for q in range(G):
                        w2_tile(hbl[q], OBp, q)
                    nc.vector.tensor_add(
                        outsb[:, up * G : (up + 1) * G, :].opt(),
                        OBp[:].opt(),
                        b2sb[:].rearrange("p (q c) -> p q c", c=OUT),
                    )

                # Rank-2 LN correction (one K=8 matmul per tile) + GELU.
                last = u == NB - 1
                OB = statp.tile([128, G, OUT], f32, tag="OB")
                hbs = []
                for q in range(G):
                    nc.tensor.matmul(
                        pszs[q][:], bmt[0:8, :], scsb[:, q, :],
                        start=False, stop=True, skip_group_check=True,
                    )
                    hb = hbp.tile([128, H], bf16, tag="hb")
                    nc.scalar.activation(
                        hb[:], pszs[q][:], AF.Gelu, bias=0.0, scale=Y[:, q : q + 1]
                    )
                    hbs.append(hb)
                    if last:
                        w2_tile(hb, OB, q)

                prev = (u, hbs, OB)
                if last:
                    nc.vector.tensor_add(
                        outsb[:, u * G : (u + 1) * G, :].opt(),
                        OB[:].opt(),
                        b2sb[:].rearrange("p (q c) -> p q c", c=OUT),
                    )
                if u == NB // 2:
                    nc.scalar.dma_start(y_v[:, 0 : NT // 2], outsb[:, 0 : NT // 2])
                elif u == NB - 2:
                    nc.scalar.dma_start(y_v[:, NT // 2 : 13 * G], outsb[:, NT // 2 : 13 * G])

            nc.scalar.dma_start(y_v[:, 13 * G :], outsb[:, 13 * G :])

    nc.finalize()
    return nc


def _get_nc():
    if "nc" not in _cache:
        _cache["nc"] = _build()
    return _cache["nc"]


def _prep_weights(ln_w, ln_b, W1, b1, W2, b2):
    W1p = ln_w[:, None] * W1                      # [1024, 256]
    s1 = W1p.sum(axis=0)                          # [256]
    c1 = ln_b @ W1 + b1                           # [256]
    # Rank-2 moving strips: tile q reads rows (2q, 2q+1) = (s1, c1); other
    # rows pair with other tiles' stats and must be zero.
    sc = np.zeros((8, G, H), np.float32)
    for q in range(G):
        sc[2 * q, q] = s1
        sc[2 * q + 1, q] = c1
    return {
        "w1p": _bf16(W1p.reshape(KC, 128, H).transpose(1, 0, 2)),
        "scq": _bf16(sc),
        "w2rep": _bf16(np.broadcast_to(W2.T, (128, OUT, H))),
        "b2g": np.broadcast_to(np.tile(b2, G), (128, G * OUT)).astype(np.float32).copy(),
        "identb": _bf16(np.eye(128)),
        "identf": np.eye(128, dtype=np.float32),
    }


def _shard_input(x_core):
    """fp32 [8192, 1024] -> bf16 K-major [128, NB, G, KC, 128] with rows
    permuted so tile t=4u+q, partition m holds row 64m+4u+q (dense input and
    output DMAs), plus the fp8 DoubleRow-padded copy with a ones column."""
    xr = x_core.reshape(128, NB, G, KC, 128)        # [m, u, q, c, p]
    xt = np.ascontiguousarray(xr.transpose(4, 1, 2, 3, 0))  # [p, u, q, c, m]
    x8 = np.zeros((128, NB, G, KC, F8PAD), ml_dtypes.float8_e4m3fn)
    x8[..., 0:128] = np.asarray(xt, dtype=ml_dtypes.float8_e4m3fn)
    x8[..., 128] = 1.0
    return _bf16(xt), x8.reshape(128, NB, G, KCP, 2, F8PAD)


def _make_in_maps(embedding, ln_w, ln_b, W1, b1, W2, b2):
    embedding = np.asarray(embedding, dtype=np.float32)
    weights = _prep_weights(
        np.asarray(ln_w, dtype=np.float32), np.asarray(ln_b, dtype=np.float32),
        np.asarray(W1, dtype=np.float32), np.asarray(b1, dtype=np.float32),
        np.asarray(W2, dtype=np.float32), np.asarray(b2, dtype=np.float32),
    )
    maps = []
    for c in range(N_CORES):
        xt, x8 = _shard_input(embedding[c * RPC : (c + 1) * RPC])
        maps.append({"xt": xt, "x8": x8, **weights})
    return maps


def kernel(embedding, ln_w, ln_b, W1, b1, W2, b2):
    from concourse.bass_utils import run_bass_kernel_spmd

    in_maps = _make_in_maps(embedding, ln_w, ln_b, W1, b1, W2, b2)
    nc = _get_nc()
    res = run_bass_kernel_spmd(nc, in_maps, core_ids=list(range(N_CORES)))
    out = np.concatenate([res.results[c]["y"] for c in range(N_CORES)], axis=0)
    return out.astype(np.float32)
